# revision 6
# baseline (speedup 1.0000x reference)
"""Trainium2 Bass kernel for GQA attention with RoPE (dense transformer).

Problem: B=2, S=2048, H=2048, 16 query heads / 4 KV heads, head_dim 128,
causal flash-style attention, fused QKV + o_proj.

Sharding (8 cores): (batch, head-group) grid. Core c handles batch c//4 and
head group c%4 (4 query heads + their shared KV head). o_proj is computed as
per-group partials reduced on host (tensor-parallel o_proj input split).

On-core layout: activations live as [feature, token] ("transposed") so the
feature contraction dims land on SBUF partitions for the PE array:
  qT/kT: projections emit [qdim, tok]; RoPE applied in this layout on DVE.
  scoresT[k_tok, q_tok] = kT.T @ qT per 128-row k-tile; softmax runs
  without max-subtraction (scores are O(5) here); exp on ACT; row sums via
  ones-vector matmuls on PE; attn@v uses v in natural [tok, d] layout
  (projection emits vT, PE-transposed once).
Causal masking: fully-masked k-tiles are skipped entirely; diagonal tiles
get an additive -1e9 triangle before exp.

Matmuls run in float32r (TF32-like, 4x the fp32 PE throughput; measured
~1.4e-4 scaled absmax per 2048-deep contraction on HW).
"""
import math

import numpy as np

import concourse.bass as bass
import concourse.mybir as mybir
import concourse.tile as tile
from concourse import bacc
from concourse.bass_utils import run_bass_kernel_spmd
from concourse.masks import make_identity

B, S, H = 2, 2048, 2048
NH, KVH, HD = 16, 4, 128
G = 4                 # head groups (= KVH); grid = G x B = 8 cores
GQ = NH // KVH        # query heads per group
QD = GQ * HD          # per-core q dim (512)
KC = H // 128         # contraction chunks for projections (16)
TC = 4                # token chunks of 512
TT = S // 128         # 128-token tiles (16)

F32 = mybir.dt.float32
F32R = mybir.dt.float32r
AF = mybir.ActivationFunctionType

_NC = None


def _emit(nc):
    xT = nc.dram_tensor("xT", [H, S], F32R, kind="ExternalInput").ap()
    wqT = nc.dram_tensor("wqT", [H, QD], F32R, kind="ExternalInput").ap()
    wkT = nc.dram_tensor("wkT", [H, HD], F32R, kind="ExternalInput").ap()
    wvT = nc.dram_tensor("wvT", [H, HD], F32R, kind="ExternalInput").ap()
    woT = nc.dram_tensor("woT", [QD, H], F32R, kind="ExternalInput").ap()
    cosT = nc.dram_tensor("cosT", [HD, S], F32, kind="ExternalInput").ap()
    sinS = nc.dram_tensor("sinS", [HD, S], F32, kind="ExternalInput").ap()
    bqkv = nc.dram_tensor("bqkv", [128, 6], F32, kind="ExternalInput").ap()
    onesd = nc.dram_tensor("onesd", [128, 128], F32R, kind="ExternalInput").ap()
    outp = nc.dram_tensor("outp", [S, H], F32, kind="ExternalOutput").ap()

    xT3 = xT.rearrange("(ko p) t -> p ko t", p=128)
    wqT3 = wqT.rearrange("(ko p) m -> p ko m", p=128)
    wkT3 = wkT.rearrange("(ko p) m -> p ko m", p=128)
    wvT3 = wvT.rearrange("(ko p) m -> p ko m", p=128)
    woT3 = woT.rearrange("(ic p) o -> p ic o", p=128)

    with tile.TileContext(nc) as tc:
        with (
            tc.tile_pool(name="persist", bufs=1) as pp,
        ):
            # ---- persistent activations ----
            qf = pp.tile([128, GQ, S], F32R)       # post-RoPE qT  (32KB/p)
            kf = pp.tile([128, S], F32R)           # post-RoPE kT  (8KB/p)
            v_sb = pp.tile([128, TT, HD], F32R)    # v natural     (8KB/p)

            # ---- constants ----
            bias_sb = pp.tile([128, 6], F32)
            nc.sync.dma_start(bias_sb[:, :], bqkv)
            mask_sb = pp.tile([128, 128], F32)
            nc.gpsimd.memset(mask_sb[:, :], 0.0)
            # scoresT[k, q]: valid where q - k >= 0, else -1e9
            nc.gpsimd.affine_select(
                out=mask_sb[:, :], in_=mask_sb[:, :],
                compare_op=mybir.AluOpType.is_ge, fill=-1e9,
                base=0, pattern=[[1, 128]], channel_multiplier=-1,
            )
            ident = pp.tile([128, 128], F32)
            make_identity(nc, ident[:, :])
            ones_col = pp.tile([128, 1], F32R)
            nc.sync.dma_start(ones_col[:, :], onesd[:, 0:1])
            ones_row = pp.tile([1, 128], F32R)
            nc.sync.dma_start(ones_row[:, :], onesd[0:1, :])

            # ================= phase B: QKV projections + RoPE ============
            with (
                tc.tile_pool(name="projw", bufs=1) as pw,
                tc.tile_pool(name="projx", bufs=16) as px,
                tc.tile_pool(name="rope", bufs=1) as pr,
                tc.tile_pool(name="psum_b", bufs=1, space="PSUM") as psb,
            ):
                wq_sb = pw.tile([128, KC, QD], F32R)
                wk_sb = pw.tile([128, KC, HD], F32R)
                wv_sb = pw.tile([128, KC, HD], F32R)
                nc.sync.dma_start(wq_sb[:, :, :], wqT3)
                nc.sync.dma_start(wk_sb[:, :, :], wkT3)
                nc.sync.dma_start(wv_sb[:, :, :], wvT3)
                cos_sb = pw.tile([128, S], F32)
                sin_sb = pw.tile([128, S], F32)
                nc.sync.dma_start(cos_sb[:, :], cosT)
                nc.sync.dma_start(sin_sb[:, :], sinS)

                for t in range(TC):
                    ts = slice(512 * t, 512 * t + 512)
                    xcs = []
                    for ko in range(KC):
                        xc = px.tile([128, 512], F32R, tag="xc")
                        nc.sync.dma_start(xc[:, :], xT3[:, ko, ts])
                        xcs.append(xc)

                    pq = [psb.tile([128, 512], F32, tag=f"pq{m}",
                                   name=f"pq{m}_{t}")
                          for m in range(GQ)]
                    pk = psb.tile([128, 512], F32, tag="pk")
                    pv = psb.tile([128, 512], F32, tag="pv")
                    for ko in range(KC):
                        st = (ko == 0)
                        sp = (ko == KC - 1)
                        for m in range(GQ):
                            nc.tensor.matmul(
                                pq[m][:, :],
                                wq_sb[:, ko, 128 * m:128 * m + 128],
                                xcs[ko][:, :], start=st, stop=sp)
                        nc.tensor.matmul(pk[:, :], wk_sb[:, ko, :],
                                         xcs[ko][:, :], start=st, stop=sp)
                        nc.tensor.matmul(pv[:, :], wv_sb[:, ko, :],
                                         xcs[ko][:, :], start=st, stop=sp)

                    # evict + bias; RoPE for q/k on DVE in [d, tok] layout
                    for m in range(GQ + 1):
                        if m < GQ:
                            raw = pr.tile([128, 512], F32, tag="raw", bufs=3)
                            nc.scalar.activation(
                                raw[:, :], pq[m][:, :], AF.Identity,
                                bias=bias_sb[:, m:m + 1])
                        else:
                            raw = pr.tile([128, 512], F32, tag="raw", bufs=3)
                            nc.scalar.activation(
                                raw[:, :], pk[:, :], AF.Identity,
                                bias=bias_sb[:, 4:5])
                        rot = pr.tile([128, 512], F32, tag="rot", bufs=2)
                        nc.vector.tensor_copy(rot[0:64, :], raw[64:128, :])
                        nc.vector.tensor_copy(rot[64:128, :], raw[0:64, :])
                        t1 = pr.tile([128, 512], F32, tag="t1", bufs=2)
                        nc.vector.tensor_mul(t1[:, :], rot[:, :],
                                             sin_sb[:, ts])
                        t2 = pr.tile([128, 512], F32, tag="t2", bufs=2)
                        nc.vector.tensor_mul(t2[:, :], raw[:, :],
                                             cos_sb[:, ts])
                        dst = (qf[:, m, ts] if m < GQ else kf[:, ts])
                        nc.vector.tensor_add(dst, t1[:, :], t2[:, :])

                    # v: evict with bias, then transpose to natural layout
                    vT_t = pr.tile([128, 512], F32, tag="vT", bufs=2)
                    nc.scalar.activation(vT_t[:, :], pv[:, :], AF.Identity,
                                         bias=bias_sb[:, 5:6])
                    for st4 in range(4):
                        ptr = psb.tile([128, 128], F32, tag="ptr")
                        nc.tensor.transpose(
                            ptr[:, :], vT_t[:, 128 * st4:128 * st4 + 128],
                            ident[:, :])
                        nc.scalar.copy(v_sb[:, 4 * t + st4, :], ptr[:, :])

            # ================= phases C+D pools ==========================
            with (
                tc.tile_pool(name="cd", bufs=1) as pd,
                tc.tile_pool(name="expp", bufs=3) as pe,
                tc.tile_pool(name="psum_d", bufs=1, space="PSUM") as psd,
            ):
                ofl = pd.tile([128, GQ, S], F32R)      # normalized attn outT
                wo_sb = pd.tile([128, GQ, H], F32R)
                nc.sync.dma_start(wo_sb[:, :, :], woT3)

                # ======== phase C: causal flash attention per head =======
                psc_cm = tc.tile_pool(name="psum_c", bufs=1, space="PSUM")
                psc = psc_cm.__enter__()
                for h in range(GQ):
                    for qc in range(TC):
                        qs = slice(512 * qc, 512 * qc + 512)
                        last = 4 * qc + 3
                        p_sum = psc.tile([1, 512], F32, tag="sum", bufs=2)
                        p_o = psc.tile([128, 512], F32, tag="o", bufs=2)
                        for j in range(4 * qc + 4):
                            if j < 4 * qc:
                                q0, n = 512 * qc, 512
                            else:
                                q0 = 128 * j
                                n = 512 * (qc + 1) - q0
                            off = q0 - 512 * qc
                            ps = psc.tile([128, 512], F32, tag="S", bufs=3)
                            nc.tensor.matmul(
                                ps[:, 0:n], kf[:, 128 * j:128 * j + 128],
                                qf[:, h, q0:q0 + n], start=True, stop=True)
                            if j >= 4 * qc:
                                nc.vector.tensor_add(
                                    ps[:, 0:128], ps[:, 0:128], mask_sb[:, :])
                            ex = pe.tile([128, 512], F32R, tag="E")
                            nc.scalar.activation(ex[:, 0:n], ps[:, 0:n],
                                                 AF.Exp)
                            nc.tensor.matmul(
                                p_sum[0:1, off:off + n], ones_col[:, :],
                                ex[:, 0:n], start=(j == 0), stop=(j == last))
                            nc.tensor.matmul(
                                p_o[:, off:off + n], v_sb[:, j, :],
                                ex[:, 0:n], start=(j == 0), stop=(j == last))
                        inv = pe.tile([1, 512], F32R, tag="inv")
                        with nc.allow_low_precision(
                                reason="f32r rounding of softmax inv-sums"):
                            nc.vector.reciprocal(inv[0:1, :], p_sum[0:1, :])
                        p_b = psc.tile([128, 512], F32, tag="b", bufs=1)
                        nc.tensor.matmul(p_b[:, :], ones_row[0:1, :],
                                         inv[0:1, :], start=True, stop=True)
                        bc = pe.tile([128, 512], F32R, tag="bc")
                        nc.scalar.copy(bc[:, :], p_b[:, :])
                        nc.vector.tensor_mul(ofl[:, h, qs], p_o[:, :],
                                             bc[:, :])

                psc_cm.__exit__(None, None, None)

                # ======== phase D: o_proj partials =======================
                psd_cm = tc.tile_pool(name="psum_d", bufs=1, space="PSUM")
                psd = psd_cm.__enter__()
                for tt in range(TT):
                    tsl = slice(128 * tt, 128 * tt + 128)
                    for oc in range(4):
                        osl = slice(512 * oc, 512 * oc + 512)
                        pf = psd.tile([128, 512], F32, tag="f", bufs=2)
                        for ic in range(GQ):
                            nc.tensor.matmul(
                                pf[:, :], ofl[:, ic, tsl],
                                wo_sb[:, ic, osl],
                                start=(ic == 0), stop=(ic == GQ - 1))
                        fo = pe.tile([128, 512], F32, tag="fo", bufs=3)
                        nc.scalar.copy(fo[:, :], pf[:, :])
                        nc.sync.dma_start(outp[tsl, osl], fo[:, :])
                psd_cm.__exit__(None, None, None)


def _build():
    global _NC
    if _NC is None:
        nc = bacc.Bacc("TRN2", target_bir_lowering=False, debug=False,
                       num_devices=8)
        _emit(nc)
        nc.compile()
        _NC = nc
    return _NC


def _prep_inputs(x, wq, bq, wk, bk, wv, bv, wo, bo, cos, sin):
    """Host-side shard + layout prep. Core c = (g, b): g = c % 4, b = c // 4."""
    inv_sqrt_d = 1.0 / math.sqrt(HD)
    f32 = np.float32
    cosT = np.ascontiguousarray(cos.T.astype(f32))
    sinS = np.ascontiguousarray(sin.T.astype(f32))
    sinS[0:HD // 2] *= -1.0

    xTb = [np.ascontiguousarray(x[b].T.astype(f32)) for b in range(B)]

    in_maps = []
    for c in range(8):
        g, b = c % G, c // G
        wq_s = wq[QD * g:QD * (g + 1), :] * inv_sqrt_d
        bq_s = bq[QD * g:QD * (g + 1)] * inv_sqrt_d
        wk_s = wk[HD * g:HD * (g + 1), :]
        bk_s = bk[HD * g:HD * (g + 1)]
        wv_s = wv[HD * g:HD * (g + 1), :]
        bv_s = bv[HD * g:HD * (g + 1)]
        bias = np.zeros((128, 6), f32)
        bias[:, 0:4] = bq_s.reshape(GQ, HD).T
        bias[:, 4] = bk_s
        bias[:, 5] = bv_s
        in_maps.append({
            "xT": xTb[b],
            "wqT": np.ascontiguousarray(wq_s.T.astype(f32)),
            "wkT": np.ascontiguousarray(wk_s.T.astype(f32)),
            "wvT": np.ascontiguousarray(wv_s.T.astype(f32)),
            "woT": np.ascontiguousarray(wo[:, QD * g:QD * (g + 1)].T
                                        .astype(f32)),
            "cosT": cosT,
            "sinS": sinS,
            "bqkv": bias,
            "onesd": np.ones((128, 128), f32),
        })
    return in_maps


def run(inputs, trace=False):
    """Returns (full_output, BassKernelResults)."""
    inputs = {k: np.asarray(v) for k, v in inputs.items()}
    nc = _build()
    in_maps = _prep_inputs(**inputs)
    res = run_bass_kernel_spmd(nc, in_maps, core_ids=list(range(8)),
                               trace=trace)
    bo = inputs["bo"].astype(np.float64)
    out = np.empty((B, S, H), np.float32)
    for b in range(B):
        acc = np.zeros((S, H), np.float64)
        for g in range(G):
            acc += res.results[G * b + g]["outp"].astype(np.float64)
        out[b] = (acc + bo).astype(np.float32)
    return out, res


def kernel(**inputs):
    return run(inputs, trace=False)[0]


# revision 7
# speedup vs baseline: 1.0905x; 1.0905x over previous
"""Trainium2 Bass kernel for GQA attention with RoPE (dense transformer).

Problem: B=2, S=2048, H=2048, 16 query heads / 4 KV heads, head_dim 128,
causal flash-style attention, fused QKV + o_proj.

Sharding (8 cores): (batch, head-group) grid. Core c handles batch c//4 and
head group c%4 (4 query heads + their shared KV head). o_proj is computed as
per-group partials reduced on host (tensor-parallel o_proj input split).

On-core layout: activations live as [feature, token] ("transposed") so the
feature contraction dims land on SBUF partitions for the PE array:
  qT/kT: projections emit [qdim, tok]; RoPE applied in this layout on DVE.
  scoresT[k_tok, q_tok] = kT.T @ qT per 128-row k-tile; softmax runs
  without max-subtraction (scores are O(5) here); exp on ACT; row sums via
  ones-vector matmuls on PE; attn@v uses v in natural [tok, d] layout
  (projection emits vT, PE-transposed once).
Causal masking: fully-masked k-tiles are skipped entirely; diagonal tiles
get an additive -1e9 triangle before exp.

Matmuls run in float32r (TF32-like, 4x the fp32 PE throughput; measured
~1.4e-4 scaled absmax per 2048-deep contraction on HW).
"""
import math

import numpy as np

import concourse.bass as bass
import concourse.mybir as mybir
import concourse.tile as tile
from concourse import bacc
from concourse.bass_utils import run_bass_kernel_spmd
from concourse.masks import make_identity

B, S, H = 2, 2048, 2048
NH, KVH, HD = 16, 4, 128
G = 4                 # head groups (= KVH); grid = G x B = 8 cores
GQ = NH // KVH        # query heads per group
QD = GQ * HD          # per-core q dim (512)
KC = H // 128         # contraction chunks for projections (16)
TC = 4                # token chunks of 512
TT = S // 128         # 128-token tiles (16)

F32 = mybir.dt.float32
F32R = mybir.dt.float32r
AF = mybir.ActivationFunctionType

_NC = None


def _emit(nc):
    xT = nc.dram_tensor("xT", [H, S], F32R, kind="ExternalInput").ap()
    wqT = nc.dram_tensor("wqT", [H, QD], F32R, kind="ExternalInput").ap()
    wkT = nc.dram_tensor("wkT", [H, HD], F32R, kind="ExternalInput").ap()
    wvT = nc.dram_tensor("wvT", [H, HD], F32R, kind="ExternalInput").ap()
    woT = nc.dram_tensor("woT", [QD, H], F32R, kind="ExternalInput").ap()
    cosT = nc.dram_tensor("cosT", [HD, S], F32, kind="ExternalInput").ap()
    sinS = nc.dram_tensor("sinS", [HD, S], F32, kind="ExternalInput").ap()
    bqkv = nc.dram_tensor("bqkv", [128, 6], F32, kind="ExternalInput").ap()
    onesd = nc.dram_tensor("onesd", [128, 128], F32R, kind="ExternalInput").ap()
    outp = nc.dram_tensor("outp", [S, H], F32, kind="ExternalOutput").ap()

    xT3 = xT.rearrange("(ko p) t -> p ko t", p=128)
    wqT3 = wqT.rearrange("(ko p) m -> p ko m", p=128)
    wkT3 = wkT.rearrange("(ko p) m -> p ko m", p=128)
    wvT3 = wvT.rearrange("(ko p) m -> p ko m", p=128)
    woT3 = woT.rearrange("(ic p) o -> p ic o", p=128)

    with tile.TileContext(nc) as tc:
        with (
            tc.tile_pool(name="persist", bufs=1) as pp,
        ):
            # ---- persistent activations ----
            qf = pp.tile([128, GQ, S], F32R)       # post-RoPE qT  (32KB/p)
            kf = pp.tile([128, S], F32R)           # post-RoPE kT  (8KB/p)
            v_sb = pp.tile([128, TT, HD], F32R)    # v natural     (8KB/p)

            # ---- constants ----
            bias_sb = pp.tile([128, 6], F32)
            nc.sync.dma_start(bias_sb[:, :], bqkv)
            mask_sb = pp.tile([128, 128], F32)
            nc.gpsimd.memset(mask_sb[:, :], 0.0)
            # scoresT[k, q]: valid where q - k >= 0, else -1e9
            nc.gpsimd.affine_select(
                out=mask_sb[:, :], in_=mask_sb[:, :],
                compare_op=mybir.AluOpType.is_ge, fill=-1e9,
                base=0, pattern=[[1, 128]], channel_multiplier=-1,
            )
            ident = pp.tile([128, 128], F32)
            make_identity(nc, ident[:, :])
            ones_mat = pp.tile([128, 128], F32R)
            nc.sync.dma_start(ones_mat[:, :], onesd[:, :])

            # ================= phase B: QKV projections + RoPE ============
            with (
                tc.tile_pool(name="projw", bufs=1) as pw,
                tc.tile_pool(name="projx", bufs=24) as px,
                tc.tile_pool(name="rope", bufs=1) as pr,
                tc.tile_pool(name="psum_b", bufs=1, space="PSUM") as psb,
            ):
                wq_sb = pw.tile([128, KC, QD], F32R)
                wk_sb = pw.tile([128, KC, HD], F32R)
                wv_sb = pw.tile([128, KC, HD], F32R)
                nc.sync.dma_start(wk_sb[:, :, :], wkT3)
                nc.sync.dma_start(wv_sb[:, :, :], wvT3)
                for ko in range(KC):
                    nc.sync.dma_start(wq_sb[:, ko, :], wqT3[:, ko, :])
                cos_sb = pw.tile([128, S], F32)
                sin_sb = pw.tile([128, S], F32)
                nc.sync.dma_start(cos_sb[:, :], cosT)
                nc.sync.dma_start(sin_sb[:, :], sinS)

                for t in range(TC):
                    ts = slice(512 * t, 512 * t + 512)
                    xcs = []
                    for ko in range(KC):
                        xc = px.tile([128, 512], F32R, tag="xc")
                        nc.sync.dma_start(xc[:, :], xT3[:, ko, ts])
                        xcs.append(xc)

                    pq = [psb.tile([128, 512], F32, tag=f"pq{m}",
                                   name=f"pq{m}_{t}")
                          for m in range(GQ)]
                    pk = psb.tile([128, 512], F32, tag="pk")
                    pv = psb.tile([128, 512], F32, tag="pv")
                    for ko in range(KC):
                        st = (ko == 0)
                        sp = (ko == KC - 1)
                        for m in range(GQ):
                            nc.tensor.matmul(
                                pq[m][:, :],
                                wq_sb[:, ko, 128 * m:128 * m + 128],
                                xcs[ko][:, :], start=st, stop=sp)
                        nc.tensor.matmul(pk[:, :], wk_sb[:, ko, :],
                                         xcs[ko][:, :], start=st, stop=sp)
                        nc.tensor.matmul(pv[:, :], wv_sb[:, ko, :],
                                         xcs[ko][:, :], start=st, stop=sp)

                    # evict + bias; RoPE for q/k on DVE in [d, tok] layout
                    for m in range(GQ + 1):
                        raw = pr.tile([128, 512], F32, tag="raw", bufs=3,
                                      name=f"raw_{t}_{m}")
                        src_ps = pq[m][:, :] if m < GQ else pk[:, :]
                        bcol = m if m < GQ else 4
                        nc.vector.tensor_scalar_add(
                            raw[:, :], src_ps, bias_sb[:, bcol:bcol + 1])
                        rot = pr.tile([128, 512], F32, tag="rot", bufs=2)
                        nc.vector.tensor_copy(rot[0:64, :], raw[64:128, :])
                        nc.vector.tensor_copy(rot[64:128, :], raw[0:64, :])
                        t1 = pr.tile([128, 512], F32, tag="t1", bufs=2)
                        nc.vector.tensor_mul(t1[:, :], rot[:, :],
                                             sin_sb[:, ts])
                        t2 = pr.tile([128, 512], F32, tag="t2", bufs=2)
                        nc.vector.tensor_mul(t2[:, :], raw[:, :],
                                             cos_sb[:, ts])
                        dst = (qf[:, m, ts] if m < GQ else kf[:, ts])
                        nc.vector.tensor_add(dst, t1[:, :], t2[:, :])

                    # v: evict with bias, then transpose to natural layout
                    vT_t = pr.tile([128, 512], F32, tag="vT", bufs=2)
                    nc.vector.tensor_scalar_add(vT_t[:, :], pv[:, :],
                                                bias_sb[:, 5:6])
                    for st4 in range(4):
                        ptr = psb.tile([128, 128], F32, tag="ptr")
                        nc.tensor.transpose(
                            ptr[:, :], vT_t[:, 128 * st4:128 * st4 + 128],
                            ident[:, :])
                        nc.vector.tensor_copy(v_sb[:, 4 * t + st4, :],
                                              ptr[:, :])

            # ================= phases C+D pools ==========================
            with (
                tc.tile_pool(name="cd", bufs=1) as pd,
                tc.tile_pool(name="expp", bufs=3) as pe,
                tc.tile_pool(name="psum_d", bufs=1, space="PSUM") as psd,
            ):
                ofl = pd.tile([128, GQ, S], F32R)      # normalized attn outT
                wo_sb = pd.tile([128, GQ, H], F32R)
                for ic in range(GQ):
                    nc.sync.dma_start(wo_sb[:, ic, :], woT3[:, ic, :])

                # ======== phase C: causal flash attention per head =======
                psc_cm = tc.tile_pool(name="psum_c", bufs=1, space="PSUM")
                psc = psc_cm.__enter__()
                for h in range(GQ):
                    for qc in range(TC):
                        qs = slice(512 * qc, 512 * qc + 512)
                        last = 4 * qc + 3
                        p_sum = psc.tile([128, 512], F32, tag="sum", bufs=2,
                                         name=f"psum_{h}_{qc}")
                        p_o = psc.tile([128, 512], F32, tag="o", bufs=2)
                        for j in range(4 * qc + 4):
                            if j < 4 * qc:
                                q0, n = 512 * qc, 512
                            else:
                                q0 = 128 * j
                                n = 512 * (qc + 1) - q0
                            off = q0 - 512 * qc
                            ps = psc.tile([128, 512], F32, tag="S", bufs=3)
                            nc.tensor.matmul(
                                ps[:, 0:n], kf[:, 128 * j:128 * j + 128],
                                qf[:, h, q0:q0 + n], start=True, stop=True)
                            if j >= 4 * qc:
                                nc.vector.tensor_add(
                                    ps[:, 0:128], ps[:, 0:128], mask_sb[:, :])
                            ex = pe.tile([128, 512], F32R, tag="E")
                            nc.scalar.activation(ex[:, 0:n], ps[:, 0:n],
                                                 AF.Exp)
                            nc.tensor.matmul(
                                p_sum[:, off:off + n], ones_mat[:, :],
                                ex[:, 0:n], start=(j == 0), stop=(j == last))
                            nc.tensor.matmul(
                                p_o[:, off:off + n], v_sb[:, j, :],
                                ex[:, 0:n], start=(j == 0), stop=(j == last))
                        bc = pe.tile([128, 512], F32R, tag="bc")
                        with nc.allow_low_precision(
                                reason="f32r rounding of softmax inv-sums"):
                            nc.vector.reciprocal(bc[:, :], p_sum[:, :])
                        nc.vector.tensor_mul(ofl[:, h, qs], p_o[:, :],
                                             bc[:, :])

                psc_cm.__exit__(None, None, None)

                # ======== phase D: o_proj partials =======================
                psd_cm = tc.tile_pool(name="psum_d", bufs=1, space="PSUM")
                psd = psd_cm.__enter__()
                for tt in range(TT):
                    tsl = slice(128 * tt, 128 * tt + 128)
                    for oc in range(4):
                        osl = slice(512 * oc, 512 * oc + 512)
                        pf = psd.tile([128, 512], F32, tag="f", bufs=2)
                        for ic in range(GQ):
                            nc.tensor.matmul(
                                pf[:, :], ofl[:, ic, tsl],
                                wo_sb[:, ic, osl],
                                start=(ic == 0), stop=(ic == GQ - 1))
                        fo = pe.tile([128, 512], F32, tag="fo", bufs=3,
                                     name=f"fo_{tt}_{oc}")
                        nc.vector.tensor_copy(fo[:, :], pf[:, :])
                        nc.sync.dma_start(outp[tsl, osl], fo[:, :])
                psd_cm.__exit__(None, None, None)


def _build():
    global _NC
    if _NC is None:
        nc = bacc.Bacc("TRN2", target_bir_lowering=False, debug=False,
                       num_devices=8)
        _emit(nc)
        nc.compile()
        _NC = nc
    return _NC


def _prep_inputs(x, wq, bq, wk, bk, wv, bv, wo, bo, cos, sin):
    """Host-side shard + layout prep. Core c = (g, b): g = c % 4, b = c // 4."""
    inv_sqrt_d = 1.0 / math.sqrt(HD)
    f32 = np.float32
    cosT = np.ascontiguousarray(cos.T.astype(f32))
    sinS = np.ascontiguousarray(sin.T.astype(f32))
    sinS[0:HD // 2] *= -1.0

    xTb = [np.ascontiguousarray(x[b].T.astype(f32)) for b in range(B)]

    in_maps = []
    for c in range(8):
        g, b = c % G, c // G
        wq_s = wq[QD * g:QD * (g + 1), :] * inv_sqrt_d
        bq_s = bq[QD * g:QD * (g + 1)] * inv_sqrt_d
        wk_s = wk[HD * g:HD * (g + 1), :]
        bk_s = bk[HD * g:HD * (g + 1)]
        wv_s = wv[HD * g:HD * (g + 1), :]
        bv_s = bv[HD * g:HD * (g + 1)]
        bias = np.zeros((128, 6), f32)
        bias[:, 0:4] = bq_s.reshape(GQ, HD).T
        bias[:, 4] = bk_s
        bias[:, 5] = bv_s
        in_maps.append({
            "xT": xTb[b],
            "wqT": np.ascontiguousarray(wq_s.T.astype(f32)),
            "wkT": np.ascontiguousarray(wk_s.T.astype(f32)),
            "wvT": np.ascontiguousarray(wv_s.T.astype(f32)),
            "woT": np.ascontiguousarray(wo[:, QD * g:QD * (g + 1)].T
                                        .astype(f32)),
            "cosT": cosT,
            "sinS": sinS,
            "bqkv": bias,
            "onesd": np.ones((128, 128), f32),
        })
    return in_maps


def run(inputs, trace=False):
    """Returns (full_output, BassKernelResults)."""
    inputs = {k: np.asarray(v) for k, v in inputs.items()}
    nc = _build()
    in_maps = _prep_inputs(**inputs)
    res = run_bass_kernel_spmd(nc, in_maps, core_ids=list(range(8)),
                               trace=trace)
    bo = inputs["bo"].astype(np.float64)
    out = np.empty((B, S, H), np.float32)
    for b in range(B):
        acc = np.zeros((S, H), np.float64)
        for g in range(G):
            acc += res.results[G * b + g]["outp"].astype(np.float64)
        out[b] = (acc + bo).astype(np.float32)
    return out, res


def kernel(**inputs):
    return run(inputs, trace=False)[0]
